# revision 2
# baseline (speedup 1.0000x reference)
"""Trainium2 Bass kernel: per-head (head_dim=128) Walsh-Hadamard transform.

Full input  : value [16384, 4096] f32  (= [tokens, 32 heads * 128])
Full output : same shape; out[t, h*128:(h+1)*128] = (H_128 @ v) / sqrt(128)

v3 design, informed by v1/v2 traces (the kernel is HBM-stream bound at
~64 MiB per core; per-SDMA-engine effective rates measured: SWDGE-plain
26.9 GB/s > HWDGE ~24.5-25 > SWDGE-cast 20.6):

  - Input: plain f32 DMAs, bulk on the SWDGE (gpsimd) ring; the first few
    chunks are issued eagerly at the very top of the program, spread over
    all three rings, to cut the observed ~9us start bubble. 12 input bufs
    let the input stream run far ahead of compute so it finishes early and
    the output stream can saturate HBM alone in the tail.
  - PE per 128x128 block: transpose in float32r mode (1.5 cyc/row vs 2.0
    for f32; bit-exact passthrough) -> PSUM; DVE copies PSUM -> SBUF
    casting to bf16; matmul bf16 (lhsT = B^T, rhs = H*scale folded) ->
    PSUM f32 (1 cyc/row).
  - ScalarE plain-copies [128,1024] f32 PSUM -> SBUF (2-bank groups).
  - Output: middle chunks alternate the two HWDGE rings; the last tiles
    use small chunks round-robined over all three rings (input is done by
    then, so SWDGE helps drain).
"""

import math

import ml_dtypes
import numpy as np

import concourse.bass as bass  # noqa: F401  (AP helpers)
import concourse.mybir as mybir
import concourse.tile as tile
from concourse import bacc
from concourse.bass_utils import run_bass_kernel_spmd

HEAD_DIM = 128
N_CORES = 8
TOKENS = 16384
HIDDEN = 4096
P = 128  # partitions / tile token rows


def _hadamard(n: int) -> np.ndarray:
    h = np.array([[1.0]], dtype=np.float64)
    while h.shape[0] < n:
        h = np.block([[h, h], [h, -h]])
    return h


def build_nc(tok_per_core: int = TOKENS // N_CORES, hidden: int = HIDDEN,
             chunk_cols: int = 2048, small_cols: int = 1024,
             head_tiles: int = 3, tail_tiles: int = 3, eager_chunks: int = 6,
             xin_bufs: int = 7, out_bufs: int = 6, xt_bufs: int = 4,
             pt_bufs: int = 3, pz_bufs: int = 2, f32r_transpose: bool = False):
    """Build the per-core Bass program."""
    g4 = 4 * HEAD_DIM   # transpose group width (1 PSUM bank as f32)
    g8 = 8 * HEAD_DIM   # matmul group width (2 PSUM banks f32)
    assert tok_per_core % P == 0 and hidden % g8 == 0
    assert chunk_cols % g8 == 0 and hidden % chunk_cols == 0
    assert small_cols % g8 == 0 and hidden % small_cols == 0
    n_tiles = tok_per_core // P
    scale = 1.0 / math.sqrt(HEAD_DIM)

    nc = bacc.Bacc("TRN2", target_bir_lowering=False)
    x = nc.dram_tensor("x", [tok_per_core, hidden], mybir.dt.float32,
                       kind="ExternalInput")
    out = nc.dram_tensor("out", [tok_per_core, hidden], mybir.dt.float32,
                         kind="ExternalOutput")
    hm_np = (_hadamard(HEAD_DIM) * scale).astype(ml_dtypes.bfloat16)
    hm = nc.inline_tensor(hm_np, "hm")
    ident = nc.inline_tensor(np.eye(HEAD_DIM, dtype=np.float32), "ident")

    tdt = mybir.dt.float32r if f32r_transpose else mybir.dt.float32

    # chunk schedule: (tile, col0, width)
    sched = []
    for i in range(n_tiles):
        w = small_cols if (i < head_tiles or i >= n_tiles - tail_tiles) \
            else chunk_cols
        for ch in range(hidden // w):
            sched.append((i, ch * w, w))

    rings3 = None  # set inside context

    with tile.TileContext(nc) as tc:
        with (
            tc.tile_pool(name="consts", bufs=1) as cpool,
            tc.tile_pool(name="xin", bufs=xin_bufs) as xpool,
            tc.tile_pool(name="xtb", bufs=xt_bufs) as xtpool,
            tc.tile_pool(name="outb", bufs=out_bufs) as opool,
            tc.tile_pool(name="pt", bufs=pt_bufs, space="PSUM") as ptpool,
            tc.tile_pool(name="pz", bufs=pz_bufs, space="PSUM") as pzpool,
        ):
            rings3 = (nc.sync, nc.scalar, nc.gpsimd)

            # consts first, on the SWDGE ring (idle at start: outputs come
            # later), so they beat the input stream and compute starts early
            hm_sb = cpool.tile([HEAD_DIM, HEAD_DIM], mybir.dt.bfloat16)
            nc.gpsimd.dma_start(hm_sb[:], hm[:])
            id_sb = cpool.tile([HEAD_DIM, HEAD_DIM], mybir.dt.float32)
            nc.gpsimd.dma_start(id_sb[:], ident[:])
            id_t = id_sb[:].bitcast(tdt) if f32r_transpose else id_sb[:]

            # eager input: first few chunks issued before the main loop.
            # All input rides the sync (SP) HWDGE ring: SP has no other
            # work, so DMA dispatch never queues behind compute (the
            # scalar ring's dispatches sit behind ACT copies in FIFO).
            x_tiles = {}
            for k in range(min(eager_chunks, len(sched))):
                i, c0, w = sched[k]
                xt_in = xpool.tile([P, chunk_cols], mybir.dt.float32)
                nc.sync.dma_start(
                    xt_in[:, :w], x[i * P:(i + 1) * P, c0:c0 + w])
                x_tiles[k] = xt_in

            out_k = 0
            for k, (i, c0, w) in enumerate(sched):
                if k in x_tiles:
                    x_tile = x_tiles.pop(k)
                else:
                    x_tile = xpool.tile([P, chunk_cols], mybir.dt.float32)
                    nc.sync.dma_start(
                        x_tile[:, :w], x[i * P:(i + 1) * P, c0:c0 + w])
                o_tile = opool.tile([P, chunk_cols], mybir.dt.float32)
                for g in range(w // g8):
                    xts = []
                    for h4 in range(2):
                        pt = ptpool.tile([P, g4], tdt)
                        for j in range(4):
                            c = g * g8 + h4 * g4 + j * HEAD_DIM
                            src = x_tile[:, c:c + HEAD_DIM]
                            if f32r_transpose:
                                src = src.bitcast(tdt)
                            nc.tensor.transpose(
                                pt[:, j * HEAD_DIM:(j + 1) * HEAD_DIM],
                                src, id_t)
                        xt_sb = xtpool.tile([P, g4], mybir.dt.bfloat16)
                        nc.vector.tensor_copy(
                            xt_sb[:],
                            pt[:].bitcast(mybir.dt.float32)
                            if f32r_transpose else pt[:])
                        xts.append(xt_sb)
                    pz = pzpool.tile([P, g8], mybir.dt.float32)
                    for j in range(8):
                        nc.tensor.matmul(
                            pz[:, j * HEAD_DIM:(j + 1) * HEAD_DIM],
                            xts[j // 4][:, (j % 4) * HEAD_DIM:
                                        (j % 4 + 1) * HEAD_DIM],
                            hm_sb[:],
                        )
                    # scale folded into hm: plain copy on ScalarE
                    nc.scalar.copy(o_tile[:, g * g8:(g + 1) * g8], pz[:])
                if i >= n_tiles - tail_tiles:
                    # input is done by now: drain outputs on SWDGE + the
                    # idle sync ring (scalar keeps doing the final copies)
                    eng = nc.gpsimd if out_k % 2 == 0 else nc.sync
                else:
                    # output rides SWDGE (fastest path for SBUF->HBM)
                    eng = nc.gpsimd
                out_k += 1
                eng.dma_start(
                    out[i * P:(i + 1) * P, c0:c0 + w], o_tile[:, :w])
    nc.finalize()
    return nc


_NC_CACHE = {}


def _get_nc(tok_per_core: int, hidden: int):
    key = (tok_per_core, hidden)
    if key not in _NC_CACHE:
        _NC_CACHE[key] = build_nc(tok_per_core, hidden)
    return _NC_CACHE[key]


def kernel(value, **_unused) -> np.ndarray:
    value = np.ascontiguousarray(np.asarray(value), dtype=np.float32)
    tokens, hidden = value.shape
    assert tokens % N_CORES == 0
    tok_per_core = tokens // N_CORES
    nc = _get_nc(tok_per_core, hidden)
    shards = np.split(value, N_CORES, axis=0)
    in_maps = [{"x": s} for s in shards]
    res = run_bass_kernel_spmd(nc, in_maps, core_ids=list(range(N_CORES)))
    return np.concatenate([r["out"] for r in res.results], axis=0)


# revision 3
# speedup vs baseline: 1.0019x; 1.0019x over previous
"""Trainium2 Bass kernel: per-head (head_dim=128) Walsh-Hadamard transform.

Full input  : value [16384, 4096] f32  (= [tokens, 32 heads * 128])
Full output : same shape; out[t, h*128:(h+1)*128] = (H_128 @ v) / sqrt(128)

Design (trace-driven; the kernel is HBM-stream bound at ~64 MiB per
core, so every engine must stay off the DMA critical path):

  - Input: plain f32 chunk DMAs, all dispatched from the sync (SP) HWDGE
    ring -- SP has no other work, so DMA dispatch never queues behind
    compute instructions.  The first chunks are issued eagerly at the top
    of the program; constants load first on the (initially idle) SWDGE
    ring so compute can start as soon as data lands.
  - PE per 128x128 head block: fp32 transpose (2 cyc/row) -> PSUM; DVE
    copies PSUM -> SBUF casting to bf16; bf16 matmul (lhsT = B^T, rhs =
    H*scale, scale folded into the bf16 Hadamard constant) -> PSUM f32 at
    1 cyc/row.  PE ends up ~59% busy, no longer the bottleneck (the fp32
    baseline was PE-bound via 4 cyc/row matmuls).
  - ScalarE plain-copies [128,1024] f32 PSUM -> SBUF (2-bank groups).
  - Output: SWDGE (gpsimd) ring, the fastest SBUF->HBM path; the last
    tiles use small chunks alternating SWDGE with the by-then-idle sync
    ring for a short drain.
"""

import math

import ml_dtypes
import numpy as np

import concourse.bass as bass  # noqa: F401  (AP helpers)
import concourse.mybir as mybir
import concourse.tile as tile
from concourse import bacc
from concourse.bass_utils import run_bass_kernel_spmd

HEAD_DIM = 128
N_CORES = 8
TOKENS = 16384
HIDDEN = 4096
P = 128  # partitions / tile token rows


def _hadamard(n: int) -> np.ndarray:
    h = np.array([[1.0]], dtype=np.float64)
    while h.shape[0] < n:
        h = np.block([[h, h], [h, -h]])
    return h


def build_nc(tok_per_core: int = TOKENS // N_CORES, hidden: int = HIDDEN,
             chunk_cols: int = 2048, small_cols: int = 1024,
             head_tiles: int = 3, tail_tiles: int = 3, eager_chunks: int = 6,
             xin_bufs: int = 7, out_bufs: int = 6, xt_bufs: int = 4,
             pt_bufs: int = 3, pz_bufs: int = 2, f32r_transpose: bool = False):
    """Build the per-core Bass program."""
    g4 = 4 * HEAD_DIM   # transpose group width (1 PSUM bank as f32)
    g8 = 8 * HEAD_DIM   # matmul group width (2 PSUM banks f32)
    assert tok_per_core % P == 0 and hidden % g8 == 0
    assert chunk_cols % g8 == 0 and hidden % chunk_cols == 0
    assert small_cols % g8 == 0 and hidden % small_cols == 0
    n_tiles = tok_per_core // P
    scale = 1.0 / math.sqrt(HEAD_DIM)

    nc = bacc.Bacc("TRN2", target_bir_lowering=False)
    x = nc.dram_tensor("x", [tok_per_core, hidden], mybir.dt.float32,
                       kind="ExternalInput")
    out = nc.dram_tensor("out", [tok_per_core, hidden], mybir.dt.float32,
                         kind="ExternalOutput")
    hm_np = (_hadamard(HEAD_DIM) * scale).astype(ml_dtypes.bfloat16)
    hm = nc.inline_tensor(hm_np, "hm")
    ident = nc.inline_tensor(np.eye(HEAD_DIM, dtype=np.float32), "ident")

    tdt = mybir.dt.float32r if f32r_transpose else mybir.dt.float32

    # chunk schedule: (tile, col0, width)
    sched = []
    for i in range(n_tiles):
        w = small_cols if (i < head_tiles or i >= n_tiles - tail_tiles) \
            else chunk_cols
        for ch in range(hidden // w):
            sched.append((i, ch * w, w))

    rings3 = None  # set inside context

    with tile.TileContext(nc) as tc:
        with (
            tc.tile_pool(name="consts", bufs=1) as cpool,
            tc.tile_pool(name="xin", bufs=xin_bufs) as xpool,
            tc.tile_pool(name="xtb", bufs=xt_bufs) as xtpool,
            tc.tile_pool(name="outb", bufs=out_bufs) as opool,
            tc.tile_pool(name="pt", bufs=pt_bufs, space="PSUM") as ptpool,
            tc.tile_pool(name="pz", bufs=pz_bufs, space="PSUM") as pzpool,
        ):
            rings3 = (nc.sync, nc.scalar, nc.gpsimd)

            # consts first, on the SWDGE ring (idle at start: outputs come
            # later), so they beat the input stream and compute starts early
            hm_sb = cpool.tile([HEAD_DIM, HEAD_DIM], mybir.dt.bfloat16)
            nc.gpsimd.dma_start(hm_sb[:], hm[:])
            id_sb = cpool.tile([HEAD_DIM, HEAD_DIM], mybir.dt.float32)
            nc.gpsimd.dma_start(id_sb[:], ident[:])
            id_t = id_sb[:].bitcast(tdt) if f32r_transpose else id_sb[:]

            # eager input: first few chunks issued before the main loop.
            # All input rides the sync (SP) HWDGE ring: SP has no other
            # work, so DMA dispatch never queues behind compute (the
            # scalar ring's dispatches sit behind ACT copies in FIFO).
            x_tiles = {}
            for k in range(min(eager_chunks, len(sched))):
                i, c0, w = sched[k]
                xt_in = xpool.tile([P, chunk_cols], mybir.dt.float32)
                nc.sync.dma_start(
                    xt_in[:, :w], x[i * P:(i + 1) * P, c0:c0 + w])
                x_tiles[k] = xt_in

            out_k = 0
            for k, (i, c0, w) in enumerate(sched):
                if k in x_tiles:
                    x_tile = x_tiles.pop(k)
                else:
                    x_tile = xpool.tile([P, chunk_cols], mybir.dt.float32)
                    nc.sync.dma_start(
                        x_tile[:, :w], x[i * P:(i + 1) * P, c0:c0 + w])
                o_tile = opool.tile([P, chunk_cols], mybir.dt.float32)
                for g in range(w // g8):
                    xts = []
                    for h4 in range(2):
                        pt = ptpool.tile([P, g4], tdt)
                        for j in range(4):
                            c = g * g8 + h4 * g4 + j * HEAD_DIM
                            src = x_tile[:, c:c + HEAD_DIM]
                            if f32r_transpose:
                                src = src.bitcast(tdt)
                            nc.tensor.transpose(
                                pt[:, j * HEAD_DIM:(j + 1) * HEAD_DIM],
                                src, id_t)
                        xt_sb = xtpool.tile([P, g4], mybir.dt.bfloat16)
                        nc.vector.tensor_copy(
                            xt_sb[:],
                            pt[:].bitcast(mybir.dt.float32)
                            if f32r_transpose else pt[:])
                        xts.append(xt_sb)
                    pz = pzpool.tile([P, g8], mybir.dt.float32)
                    for j in range(8):
                        nc.tensor.matmul(
                            pz[:, j * HEAD_DIM:(j + 1) * HEAD_DIM],
                            xts[j // 4][:, (j % 4) * HEAD_DIM:
                                        (j % 4 + 1) * HEAD_DIM],
                            hm_sb[:],
                        )
                    # scale folded into hm: plain copy on ScalarE
                    nc.scalar.copy(o_tile[:, g * g8:(g + 1) * g8], pz[:])
                if i >= n_tiles - tail_tiles:
                    # input is done by now: drain outputs on SWDGE + the
                    # idle sync ring (scalar keeps doing the final copies)
                    eng = nc.gpsimd if out_k % 2 == 0 else nc.sync
                else:
                    # output rides SWDGE (fastest path for SBUF->HBM)
                    eng = nc.gpsimd
                out_k += 1
                eng.dma_start(
                    out[i * P:(i + 1) * P, c0:c0 + w], o_tile[:, :w])
    nc.finalize()
    return nc


_NC_CACHE = {}


def _get_nc(tok_per_core: int, hidden: int):
    key = (tok_per_core, hidden)
    if key not in _NC_CACHE:
        _NC_CACHE[key] = build_nc(tok_per_core, hidden)
    return _NC_CACHE[key]


def kernel(value, **_unused) -> np.ndarray:
    value = np.ascontiguousarray(np.asarray(value), dtype=np.float32)
    tokens, hidden = value.shape
    assert tokens % N_CORES == 0
    tok_per_core = tokens // N_CORES
    nc = _get_nc(tok_per_core, hidden)
    shards = np.split(value, N_CORES, axis=0)
    in_maps = [{"x": s} for s in shards]
    res = run_bass_kernel_spmd(nc, in_maps, core_ids=list(range(N_CORES)))
    return np.concatenate([r["out"] for r in res.results], axis=0)
